# revision 7
# baseline (speedup 1.0000x reference)
"""Trainium2 Bass kernel for the nn_Attention problem (B=4, S=1024, H=32, D=128).

Sharding: zero-collective data-parallel split. Each of the 8 cores owns one
(batch, half) pair: batch b = core//2, half = core%2. A half owns 4 of the 8
query blocks of 128 tokens, paired for causal balance:
  half 0 -> blocks [7, 6, 1, 0]   half 1 -> blocks [5, 4, 3, 2]
(both sum to 18 causal block-units). Query columns are packed in DESCENDING
block order so that key-block t only needs a PREFIX of the packed columns.
Each core computes Q proj (its tokens, all heads), K/V proj (its whole batch),
causal attention and the full output projection for its tokens, then the host
scatters the 8 token-slices back into the full [B, S, HID] output.

On-chip layout is transposed ([feature, token]) so every matmul has the
contraction dim on partitions with no transposes in the hot path:
  Q.T[d, c]   = sum_k WqT[k, d] * hidT[k, c]        (lhsT = WqT tile)
  scoresT     = K.T_block.T-contract-D with Q.T     -> [ktok, qtok] in PSUM
  exp         = ScalarE Exp with scale=1/sqrt(D), additive -1e30 causal mask
  sums        = ones[128,1].T @ expT                (PE, M=1)
  attn_outT  += V_block(natural).T-contract with expT
  out.T       = WoT-contract with attn_outT, + bias
"""

import numpy as np
import ml_dtypes

import concourse.bass as bass
import concourse.tile as tile
from concourse import bacc, mybir
from concourse.bass_utils import run_bass_kernel_spmd

B, S, H, D = 4, 1024, 32, 128
HID = H * D          # 4096
ROT = D // 2         # 64
HR = ROT // 2        # 32
ROPE_BASE = 10000.0
P = 128
NT = 512             # query tokens per core
NCORES = 8
KT = HID // P        # 32 contraction tiles
NKB = S // P         # 8 key blocks
SCALE = float(D) ** -0.5

BLOCKS = [[7, 6, 1, 0], [5, 4, 3, 2]]
# Max (over the two halves) packed-column prefix width needed for key-block t.
NVMAX = [512, 512, 512, 384, 256, 256, 256, 128]
# Column window [lo, hi) per key-block where ANY core's additive mask can be
# nonzero (host asserts masks are zero outside these windows).
MWIN = [(384, 512), (256, 512), (256, 512), (256, 384),
        (128, 256), (0, 256), (0, 256), (0, 128)]

BF = mybir.dt.bfloat16
F32 = mybir.dt.float32
AF = mybir.ActivationFunctionType
bf16 = ml_dtypes.bfloat16

_PROG = None


def _build():
    nc = bacc.Bacc("TRN2", target_bir_lowering=False, debug=False,
                   num_devices=NCORES)
    dp = nc.declare_dram_parameter
    hid3 = dp("hid3", [KT, P, S], BF, False)        # [k, p, t] = hidden[b,t,k*128+p]
    hidq = dp("hidq", [P, KT, NT], BF, False)       # packed query columns
    wq4 = dp("wq4", [H, P, KT, P], BF, False)       # [h,p,k,d] = Wq[h*128+d, k*128+p]
    wo4 = dp("wo4", [KT, P, KT, P], BF, False)      # [m,p,k,d] = Wo[m*128+d, k*128+p]
    wkv3 = dp("wkv3", [KT, P, 2 * D], BF, False)    # [k,p,c] = Wkv[c, k*128+p]
    bq2 = dp("bq2", [P, H], F32, False)
    bo2 = dp("bo2", [P, KT], F32, False)
    bkv2 = dp("bkv2", [P, 2], F32, False)
    cosq = dp("cosq", [ROT, NT], F32, False)        # duplicated [c;c], packed cols
    sinq = dp("sinq", [ROT, NT], F32, False)
    cosk = dp("cosk", [ROT, S], F32, False)
    sink = dp("sink", [ROT, S], F32, False)
    maskt = dp("maskt", [P, NKB, NT], F32, False)   # additive 0/-1e30
    ident = dp("ident", [P, P], BF, False)
    outp = dp("out", [KT, P, NT], F32, True)        # [m, dd, c] = out.T slice

    with tile.TileContext(nc) as tc:
        with (
            tc.tile_pool(name="const", bufs=1) as constp,
            tc.tile_pool(name="persist", bufs=1) as persist,
        ):
            mask_sb = constp.tile([P, NKB, NT], F32)
            nc.sync.dma_start(mask_sb[:], maskt[:])
            cosq_sb = constp.tile([ROT, NT], F32, tag="cq")
            nc.sync.dma_start(cosq_sb[:], cosq[:])
            sinq_sb = constp.tile([ROT, NT], F32, tag="sq")
            nc.sync.dma_start(sinq_sb[:], sinq[:])
            cosk_sb = constp.tile([ROT, S], F32, tag="ck")
            nc.sync.dma_start(cosk_sb[:], cosk[:])
            sink_sb = constp.tile([ROT, S], F32, tag="sk")
            nc.sync.dma_start(sink_sb[:], sink[:])
            bq_sb = constp.tile([P, H], F32, tag="bq")
            nc.sync.dma_start(bq_sb[:], bq2[:])
            bo_sb = constp.tile([P, KT], F32, tag="bo")
            nc.sync.dma_start(bo_sb[:], bo2[:])
            bkv_sb = constp.tile([P, 2], F32, tag="bkv")
            nc.sync.dma_start(bkv_sb[:], bkv2[:])
            id_sb = constp.tile([P, P], BF, tag="id")
            nc.sync.dma_start(id_sb[:], ident[:])
            ones_sb = constp.tile([P, 1], BF, tag="ones")
            nc.gpsimd.memset(ones_sb[:], 1.0)

            hidq_sb = persist.tile([P, KT, NT], BF, tag="hidq")
            nc.sync.dma_start(hidq_sb[:], hidq[:])
            attn_all = persist.tile([P, KT, NT], BF, tag="attn")
            kbf = persist.tile([P, S], BF, tag="kbf")
            vnat = persist.tile([P, NKB, P], BF, tag="vnat")

            # ---------------- KV phase ----------------
            with (
                tc.tile_pool(name="hidp", bufs=4) as hidp,
                tc.tile_pool(name="wkvp", bufs=3) as wkvp,
                tc.tile_pool(name="kvtmp", bufs=1) as kvtmp,
                tc.tile_pool(name="pskv", bufs=1, space="PSUM") as pskv,
                tc.tile_pool(name="pstr", bufs=2, space="PSUM") as pstr,
            ):
                kps = [pskv.tile([P, S // 2], F32, name=f"kps{j}", tag=f"kps{j}")
                       for j in range(2)]
                vps = [pskv.tile([P, S // 2], F32, name=f"vps{j}", tag=f"vps{j}")
                       for j in range(2)]
                for k in range(KT):
                    hh = hidp.tile([P, S], BF, tag="hh")
                    nc.sync.dma_start(hh[:], hid3[k])
                    wkv_sb = wkvp.tile([P, 2 * D], BF, tag="wkv")
                    nc.sync.dma_start(wkv_sb[:], wkv3[k])
                    st, sp_ = (k == 0), (k == KT - 1)
                    for j in range(2):
                        nc.tensor.matmul(kps[j][:], wkv_sb[:, 0:D],
                                         hh[:, j * 512:(j + 1) * 512],
                                         start=st, stop=sp_)
                        nc.tensor.matmul(vps[j][:], wkv_sb[:, D:2 * D],
                                         hh[:, j * 512:(j + 1) * 512],
                                         start=st, stop=sp_)
                kfp = kvtmp.tile([P, S], F32, tag="kfp")
                vbf = kvtmp.tile([P, S], BF, tag="vbf")
                for j in range(2):
                    sl = slice(j * 512, (j + 1) * 512)
                    nc.scalar.activation(kfp[:, sl], kps[j][:], AF.Identity,
                                         bias=bkv_sb[:, 0:1])
                    nc.scalar.activation(vbf[:, sl], vps[j][:], AF.Identity,
                                         bias=bkv_sb[:, 1:2])
                # rotate-half via a partition-swapped DMA copy so every DVE op
                # has partition-aligned operands (walrus checkSBSameStartPartition)
                ksw = kvtmp.tile([ROT, S], F32, tag="ksw")
                nc.sync.dma_start(ksw[0:HR, :], kfp[HR:ROT, :])
                nc.sync.dma_start(ksw[HR:ROT, :], kfp[0:HR, :])
                m1 = kvtmp.tile([ROT, S], F32, tag="m1")
                m2 = kvtmp.tile([ROT, S], F32, tag="m2")
                nc.vector.tensor_mul(m1[:], kfp[0:ROT, :], cosk_sb[:])
                nc.vector.tensor_mul(m2[:], ksw[:], sink_sb[:])
                nc.vector.tensor_sub(kbf[0:HR, :], m1[0:HR, :], m2[0:HR, :])
                nc.vector.tensor_add(kbf[HR:ROT, :], m1[HR:ROT, :], m2[HR:ROT, :])
                nc.vector.tensor_copy(kbf[ROT:P, :], kfp[ROT:P, :])
                for t in range(NKB):
                    vt = pstr.tile([P, P], BF, tag="vt")
                    nc.tensor.transpose(vt[:], vbf[:, t * P:(t + 1) * P], id_sb[:])
                    nc.vector.tensor_copy(vnat[:, t, :], vt[:])

            # ---------------- per-head phase + output proj ----------------
            with (
                tc.tile_pool(name="wqp", bufs=2) as wqp,
                tc.tile_pool(name="qtmp", bufs=2) as qtmp,
                tc.tile_pool(name="expp", bufs=3) as expp,
                tc.tile_pool(name="nrm", bufs=2) as nrm,
                tc.tile_pool(name="outsb", bufs=2) as outsb,
                tc.tile_pool(name="psq", bufs=2, space="PSUM") as psq,
                tc.tile_pool(name="pss", bufs=2, space="PSUM") as pss,
                tc.tile_pool(name="pso", bufs=2, space="PSUM") as pso,
                tc.tile_pool(name="pssum", bufs=2, space="PSUM") as pssum,
            ):
                def emit_qproj(h):
                    wq_sb = wqp.tile([P, KT, P], BF, tag="w")
                    nc.sync.dma_start(wq_sb[:], wq4[h])
                    qp = psq.tile([P, NT], F32, tag="qp")
                    for k in range(KT):
                        nc.tensor.matmul(qp[:], wq_sb[:, k, :], hidq_sb[:, k, :],
                                         start=(k == 0), stop=(k == KT - 1))
                    qfp = qtmp.tile([P, NT], F32, tag="qfp")
                    nc.scalar.activation(qfp[:], qp[:], AF.Identity,
                                         bias=bq_sb[:, h:h + 1])
                    qsw = qtmp.tile([ROT, NT], F32, tag="qsw")
                    nc.sync.dma_start(qsw[0:HR, :], qfp[HR:ROT, :])
                    nc.sync.dma_start(qsw[HR:ROT, :], qfp[0:HR, :])
                    m1q = qtmp.tile([ROT, NT], F32, tag="m1q")
                    m2q = qtmp.tile([ROT, NT], F32, tag="m2q")
                    qr = qtmp.tile([P, NT], BF, tag="qr")
                    nc.vector.tensor_mul(m1q[:], qfp[0:ROT, :], cosq_sb[:])
                    nc.vector.tensor_mul(m2q[:], qsw[:], sinq_sb[:])
                    nc.vector.tensor_sub(qr[0:HR, :], m1q[0:HR, :], m2q[0:HR, :])
                    nc.vector.tensor_add(qr[HR:ROT, :], m1q[HR:ROT, :], m2q[HR:ROT, :])
                    nc.vector.tensor_copy(qr[ROT:P, :], qfp[ROT:P, :])
                    return qr

                def emit_attn(h, qr):
                    op = pso.tile([P, NT], F32, tag="op")
                    sums = pssum.tile([1, NT], F32, tag="sums")
                    for t in range(NKB):
                        nv = NVMAX[t]
                        sp = pss.tile([P, NT], F32, tag="sp")
                        nc.tensor.matmul(sp[:, 0:nv], kbf[:, t * P:(t + 1) * P],
                                         qr[:, 0:nv], start=True, stop=True)
                        lo, hi = MWIN[t]
                        nc.vector.tensor_add(sp[:, lo:hi], sp[:, lo:hi],
                                             mask_sb[:, t, lo:hi])
                        ex = expp.tile([P, NT], BF, tag="ex")
                        nc.scalar.activation(ex[:, 0:nv], sp[:, 0:nv], AF.Exp,
                                             scale=SCALE)
                        nc.tensor.matmul(sums[:, 0:nv], ones_sb[:], ex[:, 0:nv],
                                         start=(t == 0), stop=(t == NKB - 1))
                        nc.tensor.matmul(op[:, 0:nv], vnat[:, t, :], ex[:, 0:nv],
                                         start=(t == 0), stop=(t == NKB - 1))
                    rs = nrm.tile([1, NT], F32, tag="rs")
                    nc.vector.reciprocal_approx_fast(rs[:], sums[:])
                    rb = nrm.tile([P, NT], F32, tag="rb")
                    nc.gpsimd.partition_broadcast(rb[:], rs[:])
                    nc.vector.tensor_mul(attn_all[:, h, :], op[:], rb[:])

                prev = None
                for h in range(H):
                    qr = emit_qproj(h)
                    if prev is not None:
                        emit_attn(*prev)
                    prev = (h, qr)
                emit_attn(*prev)

                for m in range(KT):
                    wo_sb = wqp.tile([P, KT, P], BF, tag="w")
                    nc.sync.dma_start(wo_sb[:], wo4[m])
                    fp = psq.tile([P, NT], F32, tag="qp")
                    for k in range(KT):
                        nc.tensor.matmul(fp[:], wo_sb[:, k, :], attn_all[:, k, :],
                                         start=(k == 0), stop=(k == KT - 1))
                    ob = outsb.tile([P, NT], F32, tag="ob")
                    nc.scalar.activation(ob[:], fp[:], AF.Identity,
                                         bias=bo_sb[:, m:m + 1])
                    nc.sync.dma_start(outp[m], ob[:])

    nc.compile()
    return nc


def _get_prog():
    global _PROG
    if _PROG is None:
        _PROG = _build()
    return _PROG


def _qcols(hf):
    return np.concatenate([np.arange(b * P, (b + 1) * P) for b in BLOCKS[hf]])


def _prepare_inmaps(inputs):
    pos = np.asarray(inputs["position_ids"])
    hs = np.asarray(inputs["hidden_states"], np.float32)
    Wq = np.asarray(inputs["Wq"], np.float32)
    bq = np.asarray(inputs["bq"], np.float32)
    Wkv = np.asarray(inputs["Wkv"], np.float32)
    bkv = np.asarray(inputs["bkv"], np.float32)
    Wo = np.asarray(inputs["Wo"], np.float32)
    bo = np.asarray(inputs["bo"], np.float32)

    shared = {
        "wq4": np.ascontiguousarray(
            Wq.reshape(H, P, KT, P).transpose(0, 3, 2, 1)).astype(bf16),
        "wo4": np.ascontiguousarray(
            Wo.reshape(KT, P, KT, P).transpose(0, 3, 2, 1)).astype(bf16),
        "wkv3": np.ascontiguousarray(Wkv.T.reshape(KT, P, 2 * D)).astype(bf16),
        "bq2": np.ascontiguousarray(bq.reshape(H, P).T),
        "bo2": np.ascontiguousarray(bo.reshape(KT, P).T),
        "bkv2": np.ascontiguousarray(bkv.reshape(2, P).T),
        "ident": np.eye(P, dtype=bf16),
    }

    invf = (1.0 / (ROPE_BASE ** (np.arange(0, ROT, 2, dtype=np.float32)
                                 / np.float32(ROT)))).astype(np.float32)
    in_maps = []
    for c in range(NCORES):
        b, hf = c // 2, c % 2
        qc = _qcols(hf)
        posb = pos[b].astype(np.float32)
        ang = invf[:, None] * posb[None, :]          # [32, S]
        cos1 = np.cos(ang).astype(np.float32)
        sin1 = np.sin(ang).astype(np.float32)
        cos2k = np.concatenate([cos1, cos1], 0)      # [64, S]
        sin2k = np.concatenate([sin1, sin1], 0)
        hidT = np.ascontiguousarray(hs[b].T)         # [HID, S]
        kpos = (np.arange(NKB)[None, :, None] * P
                + np.arange(P)[:, None, None])       # [P, NKB, 1]
        # Causal mask is over sequence INDICES (jnp.tril in the reference),
        # not position values; qc are the packed columns' sequence indices.
        mask = np.where(kpos <= qc[None, None, :], 0.0, -1e30).astype(np.float32)
        for t in range(NKB):
            lo, hi = MWIN[t]
            assert not mask[:, t, :lo].any() and not mask[:, t, hi:NVMAX[t]].any(), \
                f"mask outside window at t={t}"
        m = dict(shared)
        m["hid3"] = np.ascontiguousarray(hidT.reshape(KT, P, S)).astype(bf16)
        m["hidq"] = np.ascontiguousarray(
            hidT[:, qc].reshape(KT, P, NT).transpose(1, 0, 2)).astype(bf16)
        m["cosq"] = np.ascontiguousarray(cos2k[:, qc])
        m["sinq"] = np.ascontiguousarray(sin2k[:, qc])
        m["cosk"] = cos2k
        m["sink"] = sin2k
        m["maskt"] = mask
        in_maps.append(m)
    return in_maps


def _assemble(results):
    out = np.empty((B, S, HID), np.float32)
    for c in range(NCORES):
        b, hf = c // 2, c % 2
        outT = np.asarray(results[c]["out"], np.float32).reshape(HID, NT)
        out[b, _qcols(hf), :] = outT.T
    return out


def _run(inputs, trace=False, **kw):
    nc = _get_prog()
    in_maps = _prepare_inmaps(inputs)
    res = run_bass_kernel_spmd(nc, in_maps, list(range(NCORES)), trace=trace, **kw)
    return _assemble(res.results), res


def kernel(**inputs):
    out, _ = _run(inputs)
    return out


# revision 11
# speedup vs baseline: 1.0197x; 1.0197x over previous
"""Trainium2 Bass kernel for the nn_Attention problem (B=4, S=1024, H=32, D=128).

Sharding: zero-collective data-parallel split. Each of the 8 cores owns one
(batch, half) pair: batch b = core//2, half = core%2. A half owns 4 of the 8
query blocks of 128 tokens, paired for causal balance:
  half 0 -> blocks [7, 6, 1, 0]   half 1 -> blocks [5, 4, 3, 2]
(both sum to 18 causal block-units). Query columns are packed in DESCENDING
block order so that key-block t only needs a PREFIX of the packed columns.
Each core computes Q proj (its tokens, all heads), K/V proj (its whole batch),
causal attention and the full output projection for its tokens, then the host
scatters the 8 token-slices back into the full [B, S, HID] output.

On-chip layout is transposed ([feature, token]) so every matmul has the
contraction dim on partitions with no transposes in the hot path:
  Q.T[d, c]   = sum_k WqT[k, d] * hidT[k, c]        (lhsT = WqT tile)
  scoresT     = K.T_block.T-contract-D with Q.T     -> [ktok, qtok] in PSUM
  exp         = ScalarE Exp with scale=1/sqrt(D), additive -1e30 causal mask
  sums        = ones[128,1].T @ expT                (PE, M=1)
  attn_outT  += V_block(natural).T-contract with expT
  out.T       = WoT-contract with attn_outT, + bias
"""

import numpy as np
import ml_dtypes

import concourse.bass as bass
import concourse.tile as tile
from concourse import bacc, mybir
from concourse.bass_utils import run_bass_kernel_spmd

B, S, H, D = 4, 1024, 32, 128
HID = H * D          # 4096
ROT = D // 2         # 64
HR = ROT // 2        # 32
ROPE_BASE = 10000.0
P = 128
NT = 512             # query tokens per core
NCORES = 8
KT = HID // P        # 32 contraction tiles
NKB = S // P         # 8 key blocks
SCALE = float(D) ** -0.5

BLOCKS = [[7, 6, 1, 0], [5, 4, 3, 2]]
# Max (over the two halves) packed-column prefix width needed for key-block t.
NVMAX = [512, 512, 512, 384, 256, 256, 256, 128]
# Column window [lo, hi) per key-block where ANY core's additive mask can be
# nonzero (host asserts masks are zero outside these windows).
MWIN = [(384, 512), (256, 512), (256, 512), (256, 384),
        (128, 256), (0, 256), (0, 256), (0, 128)]

BF = mybir.dt.bfloat16
F32 = mybir.dt.float32
AF = mybir.ActivationFunctionType
bf16 = ml_dtypes.bfloat16

_PROG = None


def _build():
    nc = bacc.Bacc("TRN2", target_bir_lowering=False, debug=False,
                   num_devices=NCORES)
    dp = nc.declare_dram_parameter
    hid3 = dp("hid3", [KT, P, S], BF, False)        # [k, p, t] = hidden[b,t,k*128+p]
    hidq = dp("hidq", [P, KT, NT], BF, False)       # packed query columns
    wq4 = dp("wq4", [H, P, KT, P], BF, False)       # [h,p,k,d] = Wq[h*128+d, k*128+p]
    wo4 = dp("wo4", [KT, P, KT, P], BF, False)      # [m,p,k,d] = Wo[m*128+d, k*128+p]
    wkv3 = dp("wkv3", [KT, P, 2 * D], BF, False)    # [k,p,c] = Wkv[c, k*128+p]
    bq2 = dp("bq2", [P, H], F32, False)
    bo2 = dp("bo2", [P, KT], F32, False)
    bkv2 = dp("bkv2", [P, 2], F32, False)
    cosq = dp("cosq", [ROT, NT], BF, False)         # duplicated [c;c], packed cols
    sinq = dp("sinq", [ROT, NT], BF, False)
    cosk = dp("cosk", [ROT, S], BF, False)
    sink = dp("sink", [ROT, S], BF, False)
    maskt = dp("maskt", [P, NKB, NT], F32, False)   # additive 0/-1e30
    ident = dp("ident", [P, P], BF, False)
    outp = dp("out", [KT, P, NT], F32, True)        # [m, dd, c] = out.T slice

    with tile.TileContext(nc) as tc:
        with (
            tc.tile_pool(name="const", bufs=1) as constp,
            tc.tile_pool(name="persist", bufs=1) as persist,
            tc.tile_pool(name="wqp", bufs=2) as wqp,
            tc.tile_pool(name="qtmp", bufs=3) as qtmp,
            tc.tile_pool(name="expp", bufs=3) as expp,
            tc.tile_pool(name="nrm", bufs=2) as nrm,
            tc.tile_pool(name="outsb", bufs=2) as outsb,
        ):
            # ---------------- KV phase (DMAs emitted first) ----------------
            attn_all = persist.tile([P, KT, NT], BF, tag="attn")
            kbf = persist.tile([P, S], BF, tag="kbf")
            vnat = persist.tile([P, NKB, P], BF, tag="vnat")
            hidq_sb = persist.tile([P, KT, NT], BF, tag="hidq")

            with tc.tile_pool(name="psq", bufs=2, space="PSUM") as psq:

                def emit_qproj(h):
                    wq_sb = wqp.tile([P, KT, P], BF, tag="w")
                    nc.sync.dma_start(wq_sb[:], wq4[h])
                    qp = psq.tile([P, NT], F32, tag="qp")
                    for k in range(KT):
                        nc.tensor.matmul(qp[:], wq_sb[:, k, :], hidq_sb[:, k, :],
                                         start=(k == 0), stop=(k == KT - 1))
                    qfp = qtmp.tile([P, NT], BF, tag="qfp")
                    # split eviction so the partition-swap DMA can start early
                    nc.scalar.activation(qfp[0:ROT, :], qp[0:ROT, :], AF.Identity,
                                         bias=bq_sb[0:ROT, h:h + 1])
                    qsw = qtmp.tile([ROT, NT], BF, tag="qsw")
                    nc.sync.dma_start(qsw[0:HR, :], qfp[HR:ROT, :])
                    nc.sync.dma_start(qsw[HR:ROT, :], qfp[0:HR, :])
                    nc.scalar.activation(qfp[ROT:P, :], qp[ROT:P, :], AF.Identity,
                                         bias=bq_sb[ROT:P, h:h + 1])
                    m1q = qtmp.tile([ROT, NT], BF, tag="m1q")
                    m2q = qtmp.tile([ROT, NT], BF, tag="m2q")
                    qr = qtmp.tile([P, NT], BF, tag="qr")
                    nc.vector.tensor_mul(m1q[:], qfp[0:ROT, :], cosq_sb[:])
                    nc.vector.tensor_mul(m2q[:], qsw[:], sinq_sb[:])
                    nc.vector.tensor_sub(qr[0:HR, :], m1q[0:HR, :], m2q[0:HR, :])
                    nc.vector.tensor_add(qr[HR:ROT, :], m1q[HR:ROT, :],
                                         m2q[HR:ROT, :])
                    nc.vector.tensor_copy(qr[ROT:P, :], qfp[ROT:P, :])
                    return qr

                qrs = {}
                with (
                    tc.tile_pool(name="hidp", bufs=6) as hidp,
                    tc.tile_pool(name="wkvp", bufs=4) as wkvp,
                    tc.tile_pool(name="kvtmp", bufs=1) as kvtmp,
                    tc.tile_pool(name="pskv", bufs=1, space="PSUM") as pskv,
                    tc.tile_pool(name="pstr", bufs=2, space="PSUM") as pstr,
                ):
                    kps = [pskv.tile([P, S // 2], F32, name=f"kps{j}",
                                     tag=f"kps{j}") for j in range(2)]
                    vps = [pskv.tile([P, S // 2], F32, name=f"vps{j}",
                                     tag=f"vps{j}") for j in range(2)]
                    hhs, wkvs = [], []
                    for k in range(KT):
                        hh = hidp.tile([P, S], BF, tag="hh")
                        nc.sync.dma_start(hh[:], hid3[k])
                        wkv_sb = wkvp.tile([P, 2 * D], BF, tag="wkv")
                        nc.sync.dma_start(wkv_sb[:], wkv3[k])
                        hhs.append(hh)
                        wkvs.append(wkv_sb)

                    # small consts needed by the KV epilogue
                    cosk_sb = constp.tile([ROT, S], BF, tag="ck")
                    nc.sync.dma_start(cosk_sb[:], cosk[:])
                    sink_sb = constp.tile([ROT, S], BF, tag="sk")
                    nc.sync.dma_start(sink_sb[:], sink[:])
                    bkv_sb = constp.tile([P, 2], F32, tag="bkv")
                    nc.sync.dma_start(bkv_sb[:], bkv2[:])
                    id_sb = constp.tile([P, P], BF, tag="id")
                    nc.sync.dma_start(id_sb[:], ident[:])
                    ones_sb = constp.tile([P, 1], BF, tag="ones")
                    nc.gpsimd.memset(ones_sb[:], 1.0)

                    # KV matmuls
                    for k in range(KT):
                        st, sp_ = (k == 0), (k == KT - 1)
                        for j in range(2):
                            nc.tensor.matmul(kps[j][:], wkvs[k][:, 0:D],
                                             hhs[k][:, j * 512:(j + 1) * 512],
                                             start=st, stop=sp_)
                            nc.tensor.matmul(vps[j][:], wkvs[k][:, D:2 * D],
                                             hhs[k][:, j * 512:(j + 1) * 512],
                                             start=st, stop=sp_)

                    # remaining inputs stream while KV matmuls run
                    for k in range(KT):
                        nc.sync.dma_start(hidq_sb[:, k, :], hidq[:, k, :])
                    cosq_sb = constp.tile([ROT, NT], BF, tag="cq")
                    nc.sync.dma_start(cosq_sb[:], cosq[:])
                    sinq_sb = constp.tile([ROT, NT], BF, tag="sq")
                    nc.sync.dma_start(sinq_sb[:], sinq[:])
                    bq_sb = constp.tile([P, H], F32, tag="bq")
                    nc.sync.dma_start(bq_sb[:], bq2[:])
                    bo_sb = constp.tile([P, KT], F32, tag="bo")
                    nc.sync.dma_start(bo_sb[:], bo2[:])
                    mask_sb = constp.tile([P, NKB, NT], F32, tag="mask")
                    for t in range(NKB):
                        nc.sync.dma_start(mask_sb[:, t, :], maskt[:, t, :])

                    # overlap first two Q projections with the KV epilogue
                    qrs[0] = emit_qproj(0)
                    qrs[1] = emit_qproj(1)

                    # ---- KV epilogue: evictions, K RoPE, V transpose ----
                    kfp = kvtmp.tile([P, S], BF, tag="kfp")
                    vbf = kvtmp.tile([P, S], BF, tag="vbf")
                    for j in range(2):
                        sl = slice(j * 512, (j + 1) * 512)
                        nc.scalar.activation(kfp[:, sl], kps[j][:], AF.Identity,
                                             bias=bkv_sb[:, 0:1])
                        nc.scalar.activation(vbf[:, sl], vps[j][:], AF.Identity,
                                             bias=bkv_sb[:, 1:2])
                    ksw = kvtmp.tile([ROT, S], BF, tag="ksw")
                    nc.sync.dma_start(ksw[0:HR, :], kfp[HR:ROT, :])
                    nc.sync.dma_start(ksw[HR:ROT, :], kfp[0:HR, :])
                    m1 = kvtmp.tile([ROT, S], BF, tag="m1")
                    m2 = kvtmp.tile([ROT, S], BF, tag="m2")
                    nc.vector.tensor_mul(m1[:], kfp[0:ROT, :], cosk_sb[:])
                    nc.vector.tensor_mul(m2[:], ksw[:], sink_sb[:])
                    nc.vector.tensor_sub(kbf[0:HR, :], m1[0:HR, :], m2[0:HR, :])
                    nc.vector.tensor_add(kbf[HR:ROT, :], m1[HR:ROT, :],
                                         m2[HR:ROT, :])
                    nc.vector.tensor_copy(kbf[ROT:P, :], kfp[ROT:P, :])
                    for t in range(NKB):
                        vt = pstr.tile([P, P], BF, tag="vt")
                        nc.tensor.transpose(vt[:], vbf[:, t * P:(t + 1) * P],
                                            id_sb[:])
                        nc.vector.tensor_copy(vnat[:, t, :], vt[:])

                with (
                    tc.tile_pool(name="pss", bufs=3, space="PSUM") as pss,
                    tc.tile_pool(name="pso", bufs=2, space="PSUM") as pso,
                    tc.tile_pool(name="pssum", bufs=1, space="PSUM") as pssum,
                ):
                    def emit_attn(h, qr):
                        op = pso.tile([P, NT], F32, tag="op")
                        sums = pssum.tile([1, NT], F32, tag="sums")
                        for t in range(NKB):
                            nv = NVMAX[t]
                            sp = pss.tile([P, NT], F32, tag="sp")
                            nc.tensor.matmul(sp[:, 0:nv],
                                             kbf[:, t * P:(t + 1) * P],
                                             qr[:, 0:nv], start=True, stop=True)
                            lo, hi = MWIN[t]
                            nc.vector.tensor_add(sp[:, lo:hi], sp[:, lo:hi],
                                                 mask_sb[:, t, lo:hi])
                            ex = expp.tile([P, NT], BF, tag="ex")
                            nc.scalar.activation(ex[:, 0:nv], sp[:, 0:nv], AF.Exp,
                                                 scale=SCALE)
                            nc.tensor.matmul(sums[:, 0:nv], ones_sb[:],
                                             ex[:, 0:nv],
                                             start=(t == 0), stop=(t == NKB - 1))
                            nc.tensor.matmul(op[:, 0:nv], vnat[:, t, :],
                                             ex[:, 0:nv],
                                             start=(t == 0), stop=(t == NKB - 1))
                        rs = nrm.tile([1, NT], F32, tag="rs")
                        nc.vector.reciprocal_approx_fast(rs[:], sums[:])
                        rb = nrm.tile([P, NT], F32, tag="rb")
                        nc.gpsimd.partition_broadcast(rb[:], rs[:])
                        nc.vector.tensor_mul(attn_all[:, h, :], op[:], rb[:])

                    # ---- steady-state head pipeline (depth 2) ----
                    for h in range(2, H):
                        qrs[h] = emit_qproj(h)
                        emit_attn(h - 2, qrs.pop(h - 2))
                    emit_attn(H - 2, qrs.pop(H - 2))
                    emit_attn(H - 1, qrs.pop(H - 1))

                    # ---- output projection ----
                    for m in range(KT):
                        wo_sb = wqp.tile([P, KT, P], BF, tag="w")
                        nc.sync.dma_start(wo_sb[:], wo4[m])
                        fp = psq.tile([P, NT], F32, tag="qp")
                        for k in range(KT):
                            nc.tensor.matmul(fp[:], wo_sb[:, k, :],
                                             attn_all[:, k, :],
                                             start=(k == 0), stop=(k == KT - 1))
                        ob = outsb.tile([P, NT], F32, tag="ob")
                        nc.scalar.activation(ob[:], fp[:], AF.Identity,
                                             bias=bo_sb[:, m:m + 1])
                        nc.sync.dma_start(outp[m], ob[:])

    nc.compile()
    return nc


def _get_prog():
    global _PROG
    if _PROG is None:
        _PROG = _build()
    return _PROG


def _qcols(hf):
    return np.concatenate([np.arange(b * P, (b + 1) * P) for b in BLOCKS[hf]])


def _prepare_inmaps(inputs):
    pos = np.asarray(inputs["position_ids"])
    hs = np.asarray(inputs["hidden_states"], np.float32)
    Wq = np.asarray(inputs["Wq"], np.float32)
    bq = np.asarray(inputs["bq"], np.float32)
    Wkv = np.asarray(inputs["Wkv"], np.float32)
    bkv = np.asarray(inputs["bkv"], np.float32)
    Wo = np.asarray(inputs["Wo"], np.float32)
    bo = np.asarray(inputs["bo"], np.float32)

    shared = {
        "wq4": np.ascontiguousarray(
            Wq.reshape(H, P, KT, P).transpose(0, 3, 2, 1)).astype(bf16),
        "wo4": np.ascontiguousarray(
            Wo.reshape(KT, P, KT, P).transpose(0, 3, 2, 1)).astype(bf16),
        "wkv3": np.ascontiguousarray(Wkv.T.reshape(KT, P, 2 * D)).astype(bf16),
        "bq2": np.ascontiguousarray(bq.reshape(H, P).T),
        "bo2": np.ascontiguousarray(bo.reshape(KT, P).T),
        "bkv2": np.ascontiguousarray(bkv.reshape(2, P).T),
        "ident": np.eye(P, dtype=bf16),
    }

    invf = (1.0 / (ROPE_BASE ** (np.arange(0, ROT, 2, dtype=np.float32)
                                 / np.float32(ROT)))).astype(np.float32)
    in_maps = []
    for c in range(NCORES):
        b, hf = c // 2, c % 2
        qc = _qcols(hf)
        posb = pos[b].astype(np.float32)
        ang = invf[:, None] * posb[None, :]          # [32, S]
        cos1 = np.cos(ang).astype(np.float32)
        sin1 = np.sin(ang).astype(np.float32)
        cos2k = np.concatenate([cos1, cos1], 0)      # [64, S]
        sin2k = np.concatenate([sin1, sin1], 0)
        hidT = np.ascontiguousarray(hs[b].T)         # [HID, S]
        kpos = (np.arange(NKB)[None, :, None] * P
                + np.arange(P)[:, None, None])       # [P, NKB, 1]
        # Causal mask is over sequence INDICES (jnp.tril in the reference),
        # not position values; qc are the packed columns' sequence indices.
        mask = np.where(kpos <= qc[None, None, :], 0.0, -1e30).astype(np.float32)
        for t in range(NKB):
            lo, hi = MWIN[t]
            assert not mask[:, t, :lo].any() and not mask[:, t, hi:NVMAX[t]].any(), \
                f"mask outside window at t={t}"
        m = dict(shared)
        m["hid3"] = np.ascontiguousarray(hidT.reshape(KT, P, S)).astype(bf16)
        m["hidq"] = np.ascontiguousarray(
            hidT[:, qc].reshape(KT, P, NT).transpose(1, 0, 2)).astype(bf16)
        m["cosq"] = np.ascontiguousarray(cos2k[:, qc]).astype(bf16)
        m["sinq"] = np.ascontiguousarray(sin2k[:, qc]).astype(bf16)
        m["cosk"] = cos2k.astype(bf16)
        m["sink"] = sin2k.astype(bf16)
        m["maskt"] = mask
        in_maps.append(m)
    return in_maps


def _assemble(results):
    out = np.empty((B, S, HID), np.float32)
    for c in range(NCORES):
        b, hf = c // 2, c % 2
        outT = np.asarray(results[c]["out"], np.float32).reshape(HID, NT)
        out[b, _qcols(hf), :] = outT.T
    return out


def _run(inputs, trace=False, **kw):
    nc = _get_prog()
    in_maps = _prepare_inmaps(inputs)
    res = run_bass_kernel_spmd(nc, in_maps, list(range(NCORES)), trace=trace, **kw)
    return _assemble(res.results), res


def kernel(**inputs):
    out, _ = _run(inputs)
    return out
